# revision 19
# baseline (speedup 1.0000x reference)
"""Trainium2 Bass kernel for a 2-layer LSTM (B=256, T=512, D=64, H=512) + FC on last step.

Sharding: data-parallel over batch — 32 samples per NeuronCore on 8 cores.
Per-core design (everything SBUF-resident, no HBM traffic inside the loop):
  - gates layout: [batch=32 partitions, 4H=2048 free] (free-dim gate packing;
    partition-packing the gates is illegal for the elementwise chain because
    DVE/ACT lanes are physically tied to partitions). Computed on PE as
    gates = state.T @ W with the (small) state as the stationary operand and
    the (large) weights as the moving operand in N=512 chunks.
  - default (version 3 = "V4" design): layer1 runs one sb-step block behind
    layer0 so the two recurrences interleave on the PE and each layer's
    ACT/DVE chain hides under the other's matmuls; layer1's input projection
    is a bulk M=128 GEMM staggered into the step loop (one m-tile per 4
    steps); gates accumulate in [32,1024] PSUM halves (psum stays within 8
    banks incl. transpose + projection tiles); i+f sigmoid merged into one
    [32,1024] activation (4 ACT instrs/step-layer instead of 5).
  - recurrent state h kept transposed ([H on partitions, batch on free]) so it
    can feed the next matmul as lhsT; rebuilt each step via 4 PE transposes.
  - biases folded in: layer0 via an appended ones-row on x.T (K=65 chunk),
    layer1 via a K=1 ones-row matmul in the projection.
  - matmuls/h in bf16, cell state c and gate nonlinearities in f32. All
    elementwise on DVE (gpsimd offload measured slower on HW).
Measured (8-core SPMD, per-exec slope timing): 8.56 ms vs 13.44 ms for the
sequential V2 schedule; local CoreSim cost model predicts 5.07 ms (PE-bound).
"""

import numpy as np
import ml_dtypes

import concourse.bass as bass
import concourse.mybir as mybir
import concourse.tile as tile
from concourse.bass_utils import run_bass_kernel_spmd
from concourse.masks import make_identity

BF16 = mybir.dt.bfloat16
F32 = mybir.dt.float32

B, T, D, H, O = 256, 512, 64, 512, 1
G = 4 * H  # 2048
NCORES = 8
BL = B // NCORES  # 32
NK_H = H // 128  # 4 K-chunks for an H-sized contraction
NN = G // 512  # 4 N-chunks of 512 gate columns
SIG = mybir.ActivationFunctionType.Sigmoid
TANH = mybir.ActivationFunctionType.Tanh


def _split_excess_waits(nc, max_waits: int = 1) -> int:
    """This container's walrus rejects >1 sync wait per instruction; move
    excess waits onto preceding same-engine NOPs (same-engine earlier wait
    is ordering-equivalent)."""
    n_split = 0
    for f in nc.m.functions:
        for bb in f.blocks:
            new_insts = []
            for inst in bb.instructions:
                si = inst.sync_info
                if si is not None and si.on_wait and len(si.on_wait) > max_waits:
                    waits = list(si.on_wait)
                    while len(waits) > max_waits:
                        chunk, waits = waits[:max_waits], waits[max_waits:]
                        nop = mybir.InstNoOp(
                            name=f"{inst.name}-wsplit-{n_split}", ins=[], outs=[]
                        )
                        nop.engine = inst.engine
                        nop.sync_info = mybir.SyncInfo(on_wait=chunk, on_update=[])
                        new_insts.append(nop)
                        n_split += 1
                    si.on_wait = waits
                new_insts.append(inst)
            bb.instructions[:] = new_insts
    return n_split


BLOCK_S = 32  # steps per layer-1 input-projection batch (V2 path)


def _default_version() -> int:
    import os

    return int(os.environ.get("LSTM_KERNEL_VERSION", "3"))


def build_lstm_nc(t_steps: int = T, version: int | None = None,
                  split_waits: bool = True):
    if version is None:
        version = _default_version()
    nc = bass.Bass("TRN2")

    xt_d = nc.dram_tensor("xt", [D + 1, t_steps, BL], BF16, kind="ExternalInput")
    w0a_d = nc.dram_tensor("w0a", [D + 1, G], BF16, kind="ExternalInput")
    w0b_d = nc.dram_tensor("w0b", [128, NK_H, G], BF16, kind="ExternalInput")
    w1_d = nc.dram_tensor("w1", [128, 2 * NK_H, G], BF16, kind="ExternalInput")
    w1bias_d = nc.dram_tensor("w1bias", [1, G], BF16, kind="ExternalInput")
    fcw_d = nc.dram_tensor("fcw", [128, NK_H], BF16, kind="ExternalInput")
    ident4_d = nc.dram_tensor("ident4", [128, BL], BF16, kind="ExternalInput")
    fcb_d = nc.dram_tensor("fcb", [1, 1], F32, kind="ExternalInput")
    y_d = nc.dram_tensor("y", [BL, O], F32, kind="ExternalOutput")

    with tile.TileContext(nc) as tc:
        with (
            tc.tile_pool(name="singles", bufs=1) as singles,
            tc.tile_pool(name="state", bufs=1) as state,
            tc.tile_pool(name="work", bufs=3) as work,
            tc.tile_pool(name="psum", bufs=8, space="PSUM") as psum,
        ):
            # --- resident constants ---
            xt_s = singles.tile([D + 1, t_steps, BL], BF16)
            nc.sync.dma_start(out=xt_s, in_=xt_d[:, :, :])
            w0a_s = singles.tile([D + 1, G], BF16)
            nc.sync.dma_start(out=w0a_s, in_=w0a_d[:, :])
            w0b_s = singles.tile([128, NK_H, G], BF16)
            nc.sync.dma_start(out=w0b_s, in_=w0b_d[:, :, :])
            w1_s = singles.tile([128, 2 * NK_H, G], BF16)
            nc.sync.dma_start(out=w1_s, in_=w1_d[:, :, :])
            w1b_s = singles.tile([1, G], BF16)
            nc.sync.dma_start(out=w1b_s, in_=w1bias_d[:, :])
            fcw_s = singles.tile([128, NK_H], BF16)
            nc.sync.dma_start(out=fcw_s, in_=fcw_d[:, :])
            fcb_s = singles.tile([BL, 1], F32)
            nc.sync.dma_start(out=fcb_s, in_=fcb_d[:, :].to_broadcast((BL, 1)))
            ident = singles.tile([BL, BL], BF16)
            make_identity(nc, ident)
            ones_r = singles.tile([1, BL], BF16)
            nc.vector.memset(ones_r, 1.0)
            ones_r128 = singles.tile([1, 128], BF16)
            nc.vector.memset(ones_r128, 1.0)
            ident4_s = singles.tile([128, BL], BF16)
            nc.sync.dma_start(out=ident4_s, in_=ident4_d[:, :])

            # --- recurrent state ---
            h0T = state.tile([128, NK_H, BL], BF16)
            h1T = state.tile([128, NK_H, BL], BF16)
            c0 = state.tile([BL, H], F32)
            c1 = state.tile([BL, H], F32)
            for st in (h0T, h1T, c0, c1):
                nc.vector.memset(st, 0.0)

            def lstm_step(t, hT, cell, w_ih_first, w_s, kslices):
                """One LSTM cell update in gates-[BL, G]-layout.

                w_ih_first: (lhsT, rhs_tile) for the leading K-chunk
                  (x+ones row for layer0 / ones-row bias for layer1 /
                  identity+xp1 inject for layer1-V2).
                kslices: list of (lhsT_tile, k_index_in_w_s) for the
                  remaining accumulation chunks.
                """
                gch = []
                for n in range(NN):
                    ns = slice(n * 512, (n + 1) * 512)
                    gn = psum.tile([BL, 512], F32, tag="ps")
                    if callable(w_ih_first):
                        lhsT0, rhs0, tpos = w_ih_first(n)
                    else:
                        lhsT0, rhs0, tpos = (
                            w_ih_first[0], w_ih_first[1][:, ns], None)
                    nc.tensor.matmul(
                        gn, lhsT0, rhs0, start=True, stop=False,
                        tile_position=tpos,
                    )
                    for j, (lhsT_k, wk) in enumerate(kslices):
                        nc.tensor.matmul(
                            gn,
                            lhsT_k,
                            w_s[:, wk, ns],
                            start=False,
                            stop=(j == len(kslices) - 1),
                        )
                    gch.append(gn)

                sig_i = work.tile([BL, 512], F32, tag="sig_i")
                sig_f = work.tile([BL, 512], F32, tag="sig_f")
                tanh_g = work.tile([BL, 512], F32, tag="tanh_g")
                sig_o = work.tile([BL, 512], F32, tag="sig_o")
                nc.scalar.activation(sig_i, gch[0], SIG)
                nc.scalar.activation(sig_f, gch[1], SIG)
                nc.scalar.activation(tanh_g, gch[2], TANH)
                nc.scalar.activation(sig_o, gch[3], SIG)

                ig = work.tile([BL, 512], F32, tag="ig")
                nc.vector.tensor_mul(ig, sig_i, tanh_g)
                nc.vector.tensor_mul(cell, cell, sig_f)
                nc.vector.tensor_add(cell, cell, ig)
                tanh_c = work.tile([BL, 512], F32, tag="tanh_c")
                nc.scalar.activation(tanh_c, cell, TANH)
                h_new = work.tile([BL, H], BF16, tag="h_new")
                nc.vector.tensor_mul(h_new, sig_o, tanh_c)

                # transpose h_new [32, 512] -> hT [128, 4, 32]
                tp = psum.tile([128, NK_H, BL], BF16, tag="ps")
                for k in range(NK_H):
                    nc.tensor.transpose(
                        tp[:, k, :], h_new[:, k * 128 : (k + 1) * 128], ident
                    )
                nc.vector.tensor_copy(hT, tp)

            if version == 1:
                for t in range(t_steps):
                    lstm_step(
                        t,
                        h0T,
                        c0,
                        (xt_s[:, t, :], w0a_s),
                        w0b_s,
                        [(h0T[:, k, :], k) for k in range(NK_H)],
                    )
                    lstm_step(
                        t,
                        h1T,
                        c1,
                        (ones_r, w1b_s),
                        w1_s,
                        [(h0T[:, k, :], k) for k in range(NK_H)]
                        + [(h1T[:, k, :], NK_H + k) for k in range(NK_H)],
                    )
            else:
                # V2: per block of BLOCK_S steps — run layer0 alone collecting
                # transposed h0 into a block buffer, bulk-GEMM layer1's input
                # projection at full M=128 PE utilization, then run layer1's
                # recurrence with the projection injected via a K=32 identity
                # matmul.
                SB = BLOCK_S
                assert t_steps % SB == 0 and SB % 4 == 0
                h0blk = state.tile([128, NK_H, SB, BL], BF16)
                xp1blk = state.tile([128, SB // 4, NN, 512], BF16)
                for b in range(t_steps // SB):
                    for s in range(SB):
                        t = b * SB + s
                        prev = (
                            h0T if s == 0
                            else h0blk[:, :, s - 1, :]
                        )
                        lstm_step(
                            t,
                            h0blk[:, :, s, :],
                            c0,
                            (xt_s[:, t, :], w0a_s),
                            w0b_s,
                            [(prev[:, k, :], k) for k in range(NK_H)],
                        )
                    nc.vector.tensor_copy(h0T, h0blk[:, :, SB - 1, :])
                    for m in range(SB // 4):
                        for n in range(NN):
                            ns = slice(n * 512, (n + 1) * 512)
                            xp = psum.tile([128, 512], F32, tag="ps")
                            nc.tensor.matmul(
                                xp, ones_r128, w1b_s[:, ns],
                                start=True, stop=False,
                            )
                            for k in range(NK_H):
                                nc.tensor.matmul(
                                    xp,
                                    h0blk[:, k, 4 * m : 4 * m + 4, :].rearrange(
                                        "p a b -> p (a b)"
                                    ),
                                    w1_s[:, k, ns],
                                    start=False,
                                    stop=(k == NK_H - 1),
                                )
                            nc.vector.tensor_copy(xp1blk[:, m, n, :], xp)
                    for s in range(SB):
                        t = b * SB + s
                        lstm_step(
                            t,
                            h1T,
                            c1,
                            lambda n, s=s: (
                                ident4_s[(s % 4) * BL : (s % 4 + 1) * BL, :],
                                xp1blk[
                                    (s % 4) * BL : (s % 4 + 1) * BL, s // 4, n, :
                                ],
                                ((s % 4) * BL, 0) if s % 4 == 3 else None,
                            ),
                            w1_s,
                            [(h1T[:, k, :], NK_H + k) for k in range(NK_H)],
                        )

            # --- fc on last h1 ---
            fcp = psum.tile([BL, O], F32, tag="ps")
            for k in range(NK_H):
                nc.tensor.matmul(
                    fcp,
                    h1T[:, k, :],
                    fcw_s[:, k : k + 1],
                    start=(k == 0),
                    stop=(k == NK_H - 1),
                )
            y_s = work.tile([BL, O], F32, tag="y")
            nc.vector.tensor_add(y_s, fcp, fcb_s)
            nc.sync.dma_start(out=y_d[:, :], in_=y_s)

    if split_waits:
        _split_excess_waits(nc)
    return nc


# ---------------------------------------------------------------------------
# V3: packed-gate layout. The 4 gates live on partition blocks of ONE psum
# tile [128, 512] (tile_position col offsets), in slot order (i, f, o, g) so
# sigmoid covers partitions 0:96 in one activation and tanh 96:128. Layer1
# runs one 32-step block behind layer0; its input projection (bulk GEMM at
# M=128) is staggered into the step loop. Elementwise in bf16 (cell state
# dtype configurable), ig/h muls offloaded to gpsimd.
# ---------------------------------------------------------------------------

SLOT_PT = (0, 1, 3, 2)  # partition-block slot -> pytorch gate index (i,f,o,g)


def build_lstm_nc_v3(t_steps: int = T, sb: int = 16, gh_bufs: int = 2,
                     split_waits: bool = True):
    """V4: free-dim gate layout [32, 2048] (lane-legal elementwise, as V2),
    with layer1 pipelined one sb-step block behind layer0, its input
    projection staggered into the step loop, gates accumulated/flushed in
    [32, 1024] psum halves, merged i+f sigmoid, and ig/h muls on gpsimd."""
    nc = bass.Bass("TRN2")
    assert t_steps % sb == 0 and sb % 4 == 0
    nb = t_steps // sb

    xt_d = nc.dram_tensor("xt", [D + 1, t_steps, BL], BF16, kind="ExternalInput")
    w0a_d = nc.dram_tensor("w0a", [D + 1, G], BF16, kind="ExternalInput")
    w0b_d = nc.dram_tensor("w0b", [128, NK_H, G], BF16, kind="ExternalInput")
    w1r_d = nc.dram_tensor("w1r", [128, NK_H, G], BF16, kind="ExternalInput")
    w1i_d = nc.dram_tensor("w1i", [128, NK_H, G], BF16, kind="ExternalInput")
    w1b_d = nc.dram_tensor("w1b", [1, G], BF16, kind="ExternalInput")
    fcw_d = nc.dram_tensor("fcw", [128, NK_H], BF16, kind="ExternalInput")
    ident4_d = nc.dram_tensor("ident4", [128, BL], BF16, kind="ExternalInput")
    fcb_d = nc.dram_tensor("fcb", [1, 1], F32, kind="ExternalInput")
    y_d = nc.dram_tensor("y", [BL, O], F32, kind="ExternalOutput")

    with tile.TileContext(nc) as tc:
        with (
            tc.tile_pool(name="singles", bufs=1) as singles,
            tc.tile_pool(name="state", bufs=1) as state,
            tc.tile_pool(name="work", bufs=2) as work,
            tc.tile_pool(name="pgh", bufs=gh_bufs, space="PSUM") as pgh,
            tc.tile_pool(name="ptp", bufs=2, space="PSUM") as ptp,
            tc.tile_pool(name="pp", bufs=2, space="PSUM") as pp,
        ):
            xt_s = singles.tile([D + 1, t_steps, BL], BF16)
            nc.sync.dma_start(out=xt_s, in_=xt_d[:, :, :])
            w0a_s = singles.tile([D + 1, G], BF16)
            nc.sync.dma_start(out=w0a_s, in_=w0a_d[:, :])
            w0b_s = singles.tile([128, NK_H, G], BF16)
            nc.sync.dma_start(out=w0b_s, in_=w0b_d[:, :, :])
            w1r_s = singles.tile([128, NK_H, G], BF16)
            nc.sync.dma_start(out=w1r_s, in_=w1r_d[:, :, :])
            w1i_s = singles.tile([128, NK_H, G], BF16)
            nc.sync.dma_start(out=w1i_s, in_=w1i_d[:, :, :])
            w1b_s = singles.tile([1, G], BF16)
            nc.sync.dma_start(out=w1b_s, in_=w1b_d[:, :])
            fcw_s = singles.tile([128, NK_H], BF16)
            nc.sync.dma_start(out=fcw_s, in_=fcw_d[:, :])
            fcb_s = singles.tile([BL, 1], F32)
            nc.sync.dma_start(out=fcb_s, in_=fcb_d[:, :].to_broadcast((BL, 1)))
            ident4_s = singles.tile([128, BL], BF16)
            nc.sync.dma_start(out=ident4_s, in_=ident4_d[:, :])
            ones_r128 = singles.tile([1, 128], BF16)
            nc.vector.memset(ones_r128, 1.0)

            h0T = state.tile([128, NK_H, BL], BF16)
            h1T = state.tile([128, NK_H, BL], BF16)
            c0 = state.tile([BL, H], F32)
            c1 = state.tile([BL, H], F32)
            h0blk0 = state.tile([128, NK_H, sb, BL], BF16)
            h0blk1 = state.tile([128, NK_H, sb, BL], BF16)
            xp1blk0 = state.tile([128, sb // 4, NN, 512], BF16)
            xp1blk1 = state.tile([128, sb // 4, NN, 512], BF16)
            h0blk = (h0blk0, h0blk1)
            xp1blk = (xp1blk0, xp1blk1)
            for st in (h0T, h1T, c0, c1):
                nc.vector.memset(st, 0.0)

            def gates_mm(first_mm, kslices, S):
                """Accumulate gates in two [32,1024] psum halves, flush each
                to SBUF S [32,2048] f32 via merged activations.
                Gate order in free dim: g | i | f | o (i,g in half 0 so ig starts early)."""
                for half in range(2):
                    gh = pgh.tile([BL, 1024], F32, tag="gh")
                    for n2 in range(2):
                        n = 2 * half + n2
                        o = gh[:, n2 * 512 : (n2 + 1) * 512]
                        first_mm(o, n)
                        ks = kslices(n)
                        for j, (lhsT_k, rhs_k) in enumerate(ks):
                            nc.tensor.matmul(
                                o, lhsT_k, rhs_k, start=False,
                                stop=(j == len(ks) - 1),
                            )
                    if half == 0:
                        nc.scalar.activation(S[:, 0:512], gh[:, 0:512], TANH)
                        nc.scalar.activation(S[:, 512:1024], gh[:, 512:1024], SIG)
                    else:
                        nc.scalar.activation(S[:, 1024:2048], gh, SIG)

            def cell_update(hT_dest, cell, S):
                ig = work.tile([BL, 512], F32, tag="ig")
                nc.vector.tensor_mul(ig, S[:, 512:1024], S[:, 0:512])
                nc.vector.tensor_mul(cell, cell, S[:, 1024:1536])
                nc.vector.tensor_add(cell, cell, ig)
                tc_t = work.tile([BL, 512], F32, tag="tc")
                nc.scalar.activation(tc_t, cell, TANH)
                h_new = work.tile([BL, H], BF16, tag="h")
                nc.vector.tensor_mul(h_new, S[:, 1536:2048], tc_t)
                tp = ptp.tile([128, NK_H, BL], BF16, tag="tp")
                for k in range(NK_H):
                    nc.tensor.transpose(
                        tp[:, k, :], h_new[:, k * 128 : (k + 1) * 128],
                        ident4_s[0:BL, :],
                    )
                nc.vector.tensor_copy(hT_dest, tp)

            def l0_step(pb, s):
                t = pb * sb + s
                if s == 0:
                    prev = h0T if pb == 0 else h0blk[(pb - 1) % 2][:, :, sb - 1, :]
                else:
                    prev = h0blk[pb % 2][:, :, s - 1, :]

                def first_mm(o, n):
                    nc.tensor.matmul(
                        o, xt_s[:, t, :], w0a_s[:, n * 512 : (n + 1) * 512],
                        start=True, stop=False,
                    )

                def kslices(n):
                    return [
                        (prev[:, k, :], w0b_s[:, k, n * 512 : (n + 1) * 512])
                        for k in range(NK_H)
                    ]

                S = work.tile([BL, G], F32, tag="S")
                gates_mm(first_mm, kslices, S)
                cell_update(h0blk[pb % 2][:, :, s, :], c0, S)

            def l1_step(db, s):
                sm = (s % 4) * BL
                m = s // 4

                def first_mm(o, n):
                    nc.tensor.matmul(
                        o, ident4_s[sm : sm + BL, :],
                        xp1blk[db % 2][sm : sm + BL, m, n, :],
                        start=True, stop=False, tile_position=(sm, 0),
                    )

                def kslices(n):
                    return [
                        (h1T[:, k, :], w1r_s[:, k, n * 512 : (n + 1) * 512])
                        for k in range(NK_H)
                    ]

                S = work.tile([BL, G], F32, tag="S")
                gates_mm(first_mm, kslices, S)
                cell_update(h1T, c1, S)

            def proj_mtile(db, m):
                for n in range(NN):
                    pp_t = pp.tile([128, 512], F32, tag="pp")
                    nc.tensor.matmul(
                        pp_t, ones_r128, w1b_s[:, n * 512 : (n + 1) * 512],
                        start=True, stop=False,
                    )
                    for k in range(NK_H):
                        nc.tensor.matmul(
                            pp_t,
                            h0blk[db % 2][:, k, 4 * m : 4 * m + 4, :].rearrange(
                                "p a b -> p (a b)"
                            ),
                            w1i_s[:, k, n * 512 : (n + 1) * 512],
                            start=False, stop=(k == NK_H - 1),
                        )
                    nc.scalar.copy(xp1blk[db % 2][:, m, n, :], pp_t)

            for pb in range(nb + 1):
                for s in range(sb):
                    if pb < nb:
                        if pb >= 1 and s % 4 == 0:
                            proj_mtile(pb - 1, s // 4)
                        l0_step(pb, s)
                        if pb >= 1:
                            l1_step(pb - 1, s)
                    else:
                        if s % 4 == 0:
                            proj_mtile(nb - 1, s // 4)
                        l1_step(nb - 1, s)

            fcpt = pp.tile([128, 512], F32, tag="pp")
            fcp = fcpt[0:BL, 0:O]
            for k in range(NK_H):
                nc.tensor.matmul(
                    fcp, h1T[:, k, :], fcw_s[:, k : k + 1],
                    start=(k == 0), stop=(k == NK_H - 1),
                )
            y_s = work.tile([BL, O], F32, tag="y")
            nc.vector.tensor_add(y_s, fcp, fcb_s)
            nc.sync.dma_start(out=y_d[:, :], in_=y_s)

    if split_waits:
        _split_excess_waits(nc)
    return nc


def prep_inputs_v3(x, w_ih_0, w_hh_0, b_ih_0, b_hh_0, w_ih_1, w_hh_1, b_ih_1,
                   b_hh_1, fc_w, fc_b, t_steps: int = T):
    bf = ml_dtypes.bfloat16

    def gperm(a2d):
        # last axis 2048 in pytorch gate order (i,f,g,o) -> (g,i,f,o)
        K_ = a2d.shape[0]
        return np.ascontiguousarray(
            a2d.reshape(K_, 4, 512)[:, (2, 0, 1, 3), :].reshape(K_, G)
        )

    def kchunked(w2dT):
        # [512, 2048] -> [128, 4k, 2048]
        return np.ascontiguousarray(
            gperm(w2dT).reshape(NK_H, 128, G).transpose(1, 0, 2)
        )

    w0a = gperm(np.concatenate(
        [w_ih_0.T, (b_ih_0 + b_hh_0)[None, :]], axis=0
    )).astype(bf)  # [65, 2048]
    w0b = kchunked(w_hh_0.T).astype(bf)
    w1r = kchunked(w_hh_1.T).astype(bf)
    w1i = kchunked(w_ih_1.T).astype(bf)
    w1b = gperm((b_ih_1 + b_hh_1)[None, :]).astype(bf)  # [1, 2048]
    fcw = np.ascontiguousarray(fc_w.reshape(NK_H, 128).T).astype(bf)
    fcb = fc_b.reshape(1, 1).astype(np.float32)
    ident4 = np.concatenate([np.eye(BL, dtype=np.float32)] * 4, axis=0).astype(bf)

    in_maps = []
    for c in range(NCORES):
        xc = x[c * BL : (c + 1) * BL, :t_steps, :]
        xt = np.transpose(xc, (2, 1, 0))
        xt = np.concatenate([xt, np.ones((1, t_steps, BL), np.float32)], axis=0)
        in_maps.append(
            {
                "xt": np.ascontiguousarray(xt).astype(bf),
                "w0a": w0a,
                "w0b": w0b,
                "w1r": w1r,
                "w1i": w1i,
                "w1b": w1b,
                "fcw": fcw,
                "fcb": fcb,
                "ident4": ident4,
            }
        )
    return in_maps


def prep_inputs(x, w_ih_0, w_hh_0, b_ih_0, b_hh_0, w_ih_1, w_hh_1, b_ih_1, b_hh_1,
                fc_w, fc_b, t_steps: int = T):
    """Host-side layout prep + sharding. Returns per-core in_maps."""
    bf = ml_dtypes.bfloat16
    w0a = np.concatenate(
        [w_ih_0.T, (b_ih_0 + b_hh_0)[None, :]], axis=0
    ).astype(bf)  # [65, G]
    w0b = np.ascontiguousarray(
        w_hh_0.T.reshape(NK_H, 128, G).transpose(1, 0, 2)
    ).astype(bf)  # [128, 4, G]
    w1 = np.ascontiguousarray(
        np.concatenate([w_ih_1.T, w_hh_1.T], axis=0)
        .reshape(2 * NK_H, 128, G)
        .transpose(1, 0, 2)
    ).astype(bf)  # [128, 8, G]
    w1bias = (b_ih_1 + b_hh_1)[None, :].astype(bf)  # [1, G]
    fcw = np.ascontiguousarray(fc_w.reshape(NK_H, 128).T).astype(bf)  # [128, 4]
    fcb = fc_b.reshape(1, 1).astype(np.float32)
    ident4 = np.concatenate([np.eye(BL, dtype=np.float32)] * 4, axis=0).astype(bf)

    in_maps = []
    for c in range(NCORES):
        xc = x[c * BL : (c + 1) * BL, :t_steps, :]  # [32, T, 64]
        xt = np.transpose(xc, (2, 1, 0))  # [64, T, 32]
        xt = np.concatenate([xt, np.ones((1, t_steps, BL), np.float32)], axis=0)
        in_maps.append(
            {
                "xt": np.ascontiguousarray(xt).astype(bf),
                "w0a": w0a,
                "w0b": w0b,
                "w1": w1,
                "w1bias": w1bias,
                "fcw": fcw,
                "fcb": fcb,
                "ident4": ident4,
            }
        )
    return in_maps


_NC_CACHE = {}


def kernel(x, w_ih_0, w_hh_0, b_ih_0, b_hh_0, w_ih_1, w_hh_1, b_ih_1, b_hh_1,
           fc_w, fc_b):
    x = np.asarray(x, np.float32)
    args = [np.asarray(a, np.float32) for a in (
        w_ih_0, w_hh_0, b_ih_0, b_hh_0, w_ih_1, w_hh_1, b_ih_1, b_hh_1, fc_w, fc_b)]
    version = _default_version()
    key = (T, version)
    if key not in _NC_CACHE:
        _NC_CACHE[key] = (
            build_lstm_nc_v3(T) if version >= 3 else build_lstm_nc(T, version=version)
        )
    nc = _NC_CACHE[key]
    prep = prep_inputs_v3 if version >= 3 else prep_inputs
    in_maps = prep(x, *args, t_steps=T)
    res = run_bass_kernel_spmd(nc, in_maps, core_ids=list(range(NCORES)))
    return np.concatenate([res.results[c]["y"] for c in range(NCORES)], axis=0)



# revision 20
# speedup vs baseline: 1.0024x; 1.0024x over previous
"""Trainium2 Bass kernel for a 2-layer LSTM (B=256, T=512, D=64, H=512) + FC on last step.

Sharding: data-parallel over batch — 32 samples per NeuronCore on 8 cores.
Per-core design (everything SBUF-resident, no HBM traffic inside the loop):
  - gates layout: [batch=32 partitions, 4H=2048 free] (free-dim gate packing;
    partition-packing the gates is illegal for the elementwise chain because
    DVE/ACT lanes are physically tied to partitions). Computed on PE as
    gates = state.T @ W with the (small) state as the stationary operand and
    the (large) weights as the moving operand in N=512 chunks.
  - default (version 3 = "V4" design): layer1 runs one sb-step block behind
    layer0 so the two recurrences interleave on the PE and each layer's
    ACT/DVE chain hides under the other's matmuls; layer1's input projection
    is a bulk M=128 GEMM staggered into the step loop (one m-tile per 4
    steps); gates accumulate in [32,1024] PSUM halves (psum stays within 8
    banks incl. transpose + projection tiles); i+f sigmoid merged into one
    [32,1024] activation (4 ACT instrs/step-layer instead of 5).
  - recurrent state h kept transposed ([H on partitions, batch on free]) so it
    can feed the next matmul as lhsT; rebuilt each step via 4 PE transposes.
  - biases folded in: layer0 via an appended ones-row on x.T (K=65 chunk),
    layer1 via a K=1 ones-row matmul in the projection.
  - matmuls/h in bf16, cell state c and gate nonlinearities in f32. All
    elementwise on DVE (gpsimd offload measured slower on HW).
Measured (8-core SPMD, per-exec slope timing): 8.56 ms vs 13.44 ms for the
sequential V2 schedule; local CoreSim cost model predicts 5.07 ms (PE-bound).
"""

import numpy as np
import ml_dtypes

import concourse.bass as bass
import concourse.mybir as mybir
import concourse.tile as tile
from concourse.bass_utils import run_bass_kernel_spmd
from concourse.masks import make_identity

BF16 = mybir.dt.bfloat16
F32 = mybir.dt.float32

B, T, D, H, O = 256, 512, 64, 512, 1
G = 4 * H  # 2048
NCORES = 8
BL = B // NCORES  # 32
NK_H = H // 128  # 4 K-chunks for an H-sized contraction
NN = G // 512  # 4 N-chunks of 512 gate columns
SIG = mybir.ActivationFunctionType.Sigmoid
TANH = mybir.ActivationFunctionType.Tanh


def _split_excess_waits(nc, max_waits: int = 1) -> int:
    """This container's walrus rejects >1 sync wait per instruction; move
    excess waits onto preceding same-engine NOPs (same-engine earlier wait
    is ordering-equivalent)."""
    n_split = 0
    for f in nc.m.functions:
        for bb in f.blocks:
            new_insts = []
            for inst in bb.instructions:
                si = inst.sync_info
                if si is not None and si.on_wait and len(si.on_wait) > max_waits:
                    waits = list(si.on_wait)
                    while len(waits) > max_waits:
                        chunk, waits = waits[:max_waits], waits[max_waits:]
                        nop = mybir.InstNoOp(
                            name=f"{inst.name}-wsplit-{n_split}", ins=[], outs=[]
                        )
                        nop.engine = inst.engine
                        nop.sync_info = mybir.SyncInfo(on_wait=chunk, on_update=[])
                        new_insts.append(nop)
                        n_split += 1
                    si.on_wait = waits
                new_insts.append(inst)
            bb.instructions[:] = new_insts
    return n_split


BLOCK_S = 32  # steps per layer-1 input-projection batch (V2 path)


def _default_version() -> int:
    import os

    return int(os.environ.get("LSTM_KERNEL_VERSION", "3"))


def build_lstm_nc(t_steps: int = T, version: int | None = None,
                  split_waits: bool = True):
    if version is None:
        version = _default_version()
    nc = bass.Bass("TRN2")

    xt_d = nc.dram_tensor("xt", [D + 1, t_steps, BL], BF16, kind="ExternalInput")
    w0a_d = nc.dram_tensor("w0a", [D + 1, G], BF16, kind="ExternalInput")
    w0b_d = nc.dram_tensor("w0b", [128, NK_H, G], BF16, kind="ExternalInput")
    w1_d = nc.dram_tensor("w1", [128, 2 * NK_H, G], BF16, kind="ExternalInput")
    w1bias_d = nc.dram_tensor("w1bias", [1, G], BF16, kind="ExternalInput")
    fcw_d = nc.dram_tensor("fcw", [128, NK_H], BF16, kind="ExternalInput")
    ident4_d = nc.dram_tensor("ident4", [128, BL], BF16, kind="ExternalInput")
    fcb_d = nc.dram_tensor("fcb", [1, 1], F32, kind="ExternalInput")
    y_d = nc.dram_tensor("y", [BL, O], F32, kind="ExternalOutput")

    with tile.TileContext(nc) as tc:
        with (
            tc.tile_pool(name="singles", bufs=1) as singles,
            tc.tile_pool(name="state", bufs=1) as state,
            tc.tile_pool(name="work", bufs=3) as work,
            tc.tile_pool(name="psum", bufs=8, space="PSUM") as psum,
        ):
            # --- resident constants ---
            xt_s = singles.tile([D + 1, t_steps, BL], BF16)
            nc.sync.dma_start(out=xt_s, in_=xt_d[:, :, :])
            w0a_s = singles.tile([D + 1, G], BF16)
            nc.sync.dma_start(out=w0a_s, in_=w0a_d[:, :])
            w0b_s = singles.tile([128, NK_H, G], BF16)
            nc.sync.dma_start(out=w0b_s, in_=w0b_d[:, :, :])
            w1_s = singles.tile([128, 2 * NK_H, G], BF16)
            nc.sync.dma_start(out=w1_s, in_=w1_d[:, :, :])
            w1b_s = singles.tile([1, G], BF16)
            nc.sync.dma_start(out=w1b_s, in_=w1bias_d[:, :])
            fcw_s = singles.tile([128, NK_H], BF16)
            nc.sync.dma_start(out=fcw_s, in_=fcw_d[:, :])
            fcb_s = singles.tile([BL, 1], F32)
            nc.sync.dma_start(out=fcb_s, in_=fcb_d[:, :].to_broadcast((BL, 1)))
            ident = singles.tile([BL, BL], BF16)
            make_identity(nc, ident)
            ones_r = singles.tile([1, BL], BF16)
            nc.vector.memset(ones_r, 1.0)
            ones_r128 = singles.tile([1, 128], BF16)
            nc.vector.memset(ones_r128, 1.0)
            ident4_s = singles.tile([128, BL], BF16)
            nc.sync.dma_start(out=ident4_s, in_=ident4_d[:, :])

            # --- recurrent state ---
            h0T = state.tile([128, NK_H, BL], BF16)
            h1T = state.tile([128, NK_H, BL], BF16)
            c0 = state.tile([BL, H], F32)
            c1 = state.tile([BL, H], F32)
            for st in (h0T, h1T, c0, c1):
                nc.vector.memset(st, 0.0)

            def lstm_step(t, hT, cell, w_ih_first, w_s, kslices):
                """One LSTM cell update in gates-[BL, G]-layout.

                w_ih_first: (lhsT, rhs_tile) for the leading K-chunk
                  (x+ones row for layer0 / ones-row bias for layer1 /
                  identity+xp1 inject for layer1-V2).
                kslices: list of (lhsT_tile, k_index_in_w_s) for the
                  remaining accumulation chunks.
                """
                gch = []
                for n in range(NN):
                    ns = slice(n * 512, (n + 1) * 512)
                    gn = psum.tile([BL, 512], F32, tag="ps")
                    if callable(w_ih_first):
                        lhsT0, rhs0, tpos = w_ih_first(n)
                    else:
                        lhsT0, rhs0, tpos = (
                            w_ih_first[0], w_ih_first[1][:, ns], None)
                    nc.tensor.matmul(
                        gn, lhsT0, rhs0, start=True, stop=False,
                        tile_position=tpos,
                    )
                    for j, (lhsT_k, wk) in enumerate(kslices):
                        nc.tensor.matmul(
                            gn,
                            lhsT_k,
                            w_s[:, wk, ns],
                            start=False,
                            stop=(j == len(kslices) - 1),
                        )
                    gch.append(gn)

                sig_i = work.tile([BL, 512], F32, tag="sig_i")
                sig_f = work.tile([BL, 512], F32, tag="sig_f")
                tanh_g = work.tile([BL, 512], F32, tag="tanh_g")
                sig_o = work.tile([BL, 512], F32, tag="sig_o")
                nc.scalar.activation(sig_i, gch[0], SIG)
                nc.scalar.activation(sig_f, gch[1], SIG)
                nc.scalar.activation(tanh_g, gch[2], TANH)
                nc.scalar.activation(sig_o, gch[3], SIG)

                ig = work.tile([BL, 512], F32, tag="ig")
                nc.vector.tensor_mul(ig, sig_i, tanh_g)
                nc.vector.tensor_mul(cell, cell, sig_f)
                nc.vector.tensor_add(cell, cell, ig)
                tanh_c = work.tile([BL, 512], F32, tag="tanh_c")
                nc.scalar.activation(tanh_c, cell, TANH)
                h_new = work.tile([BL, H], BF16, tag="h_new")
                nc.vector.tensor_mul(h_new, sig_o, tanh_c)

                # transpose h_new [32, 512] -> hT [128, 4, 32]
                tp = psum.tile([128, NK_H, BL], BF16, tag="ps")
                for k in range(NK_H):
                    nc.tensor.transpose(
                        tp[:, k, :], h_new[:, k * 128 : (k + 1) * 128], ident
                    )
                nc.vector.tensor_copy(hT, tp)

            if version == 1:
                for t in range(t_steps):
                    lstm_step(
                        t,
                        h0T,
                        c0,
                        (xt_s[:, t, :], w0a_s),
                        w0b_s,
                        [(h0T[:, k, :], k) for k in range(NK_H)],
                    )
                    lstm_step(
                        t,
                        h1T,
                        c1,
                        (ones_r, w1b_s),
                        w1_s,
                        [(h0T[:, k, :], k) for k in range(NK_H)]
                        + [(h1T[:, k, :], NK_H + k) for k in range(NK_H)],
                    )
            else:
                # V2: per block of BLOCK_S steps — run layer0 alone collecting
                # transposed h0 into a block buffer, bulk-GEMM layer1's input
                # projection at full M=128 PE utilization, then run layer1's
                # recurrence with the projection injected via a K=32 identity
                # matmul.
                SB = BLOCK_S
                assert t_steps % SB == 0 and SB % 4 == 0
                h0blk = state.tile([128, NK_H, SB, BL], BF16)
                xp1blk = state.tile([128, SB // 4, NN, 512], BF16)
                for b in range(t_steps // SB):
                    for s in range(SB):
                        t = b * SB + s
                        prev = (
                            h0T if s == 0
                            else h0blk[:, :, s - 1, :]
                        )
                        lstm_step(
                            t,
                            h0blk[:, :, s, :],
                            c0,
                            (xt_s[:, t, :], w0a_s),
                            w0b_s,
                            [(prev[:, k, :], k) for k in range(NK_H)],
                        )
                    nc.vector.tensor_copy(h0T, h0blk[:, :, SB - 1, :])
                    for m in range(SB // 4):
                        for n in range(NN):
                            ns = slice(n * 512, (n + 1) * 512)
                            xp = psum.tile([128, 512], F32, tag="ps")
                            nc.tensor.matmul(
                                xp, ones_r128, w1b_s[:, ns],
                                start=True, stop=False,
                            )
                            for k in range(NK_H):
                                nc.tensor.matmul(
                                    xp,
                                    h0blk[:, k, 4 * m : 4 * m + 4, :].rearrange(
                                        "p a b -> p (a b)"
                                    ),
                                    w1_s[:, k, ns],
                                    start=False,
                                    stop=(k == NK_H - 1),
                                )
                            nc.vector.tensor_copy(xp1blk[:, m, n, :], xp)
                    for s in range(SB):
                        t = b * SB + s
                        lstm_step(
                            t,
                            h1T,
                            c1,
                            lambda n, s=s: (
                                ident4_s[(s % 4) * BL : (s % 4 + 1) * BL, :],
                                xp1blk[
                                    (s % 4) * BL : (s % 4 + 1) * BL, s // 4, n, :
                                ],
                                ((s % 4) * BL, 0) if s % 4 == 3 else None,
                            ),
                            w1_s,
                            [(h1T[:, k, :], NK_H + k) for k in range(NK_H)],
                        )

            # --- fc on last h1 ---
            fcp = psum.tile([BL, O], F32, tag="ps")
            for k in range(NK_H):
                nc.tensor.matmul(
                    fcp,
                    h1T[:, k, :],
                    fcw_s[:, k : k + 1],
                    start=(k == 0),
                    stop=(k == NK_H - 1),
                )
            y_s = work.tile([BL, O], F32, tag="y")
            nc.vector.tensor_add(y_s, fcp, fcb_s)
            nc.sync.dma_start(out=y_d[:, :], in_=y_s)

    if split_waits:
        _split_excess_waits(nc)
    return nc


# ---------------------------------------------------------------------------
# V3: packed-gate layout. The 4 gates live on partition blocks of ONE psum
# tile [128, 512] (tile_position col offsets), in slot order (i, f, o, g) so
# sigmoid covers partitions 0:96 in one activation and tanh 96:128. Layer1
# runs one 32-step block behind layer0; its input projection (bulk GEMM at
# M=128) is staggered into the step loop. Elementwise in bf16 (cell state
# dtype configurable), ig/h muls offloaded to gpsimd.
# ---------------------------------------------------------------------------

SLOT_PT = (0, 1, 3, 2)  # partition-block slot -> pytorch gate index (i,f,o,g)


def build_lstm_nc_v3(t_steps: int = T, sb: int = 16, gh_bufs: int = 2,
                     split_waits: bool = True):
    """V4: free-dim gate layout [32, 2048] (lane-legal elementwise, as V2),
    with layer1 pipelined one sb-step block behind layer0, its input
    projection staggered into the step loop, gates accumulated/flushed in
    [32, 1024] psum halves, merged i+f sigmoid, and ig/h muls on gpsimd."""
    nc = bass.Bass("TRN2")
    assert t_steps % sb == 0 and sb % 4 == 0
    nb = t_steps // sb

    xt_d = nc.dram_tensor("xt", [D + 1, t_steps, BL], BF16, kind="ExternalInput")
    w0a_d = nc.dram_tensor("w0a", [D + 1, G], BF16, kind="ExternalInput")
    w0b_d = nc.dram_tensor("w0b", [128, NK_H, G], BF16, kind="ExternalInput")
    w1r_d = nc.dram_tensor("w1r", [128, NK_H, G], BF16, kind="ExternalInput")
    w1i_d = nc.dram_tensor("w1i", [128, NK_H, G], BF16, kind="ExternalInput")
    w1b_d = nc.dram_tensor("w1b", [1, G], BF16, kind="ExternalInput")
    fcw_d = nc.dram_tensor("fcw", [128, NK_H], BF16, kind="ExternalInput")
    ident4_d = nc.dram_tensor("ident4", [128, BL], BF16, kind="ExternalInput")
    fcb_d = nc.dram_tensor("fcb", [1, 1], F32, kind="ExternalInput")
    y_d = nc.dram_tensor("y", [BL, O], F32, kind="ExternalOutput")

    with tile.TileContext(nc) as tc:
        with (
            tc.tile_pool(name="singles", bufs=1) as singles,
            tc.tile_pool(name="state", bufs=1) as state,
            tc.tile_pool(name="work", bufs=2) as work,
            tc.tile_pool(name="pgh", bufs=gh_bufs, space="PSUM") as pgh,
            tc.tile_pool(name="ptp", bufs=2, space="PSUM") as ptp,
            tc.tile_pool(name="pp", bufs=2, space="PSUM") as pp,
        ):
            xt_s = singles.tile([D + 1, t_steps, BL], BF16)
            nc.sync.dma_start(out=xt_s, in_=xt_d[:, :, :])
            w0a_s = singles.tile([D + 1, G], BF16)
            nc.sync.dma_start(out=w0a_s, in_=w0a_d[:, :])
            w0b_s = singles.tile([128, NK_H, G], BF16)
            nc.sync.dma_start(out=w0b_s, in_=w0b_d[:, :, :])
            w1r_s = singles.tile([128, NK_H, G], BF16)
            nc.sync.dma_start(out=w1r_s, in_=w1r_d[:, :, :])
            w1i_s = singles.tile([128, NK_H, G], BF16)
            nc.sync.dma_start(out=w1i_s, in_=w1i_d[:, :, :])
            w1b_s = singles.tile([1, G], BF16)
            nc.sync.dma_start(out=w1b_s, in_=w1b_d[:, :])
            fcw_s = singles.tile([128, NK_H], BF16)
            nc.sync.dma_start(out=fcw_s, in_=fcw_d[:, :])
            fcb_s = singles.tile([BL, 1], F32)
            nc.sync.dma_start(out=fcb_s, in_=fcb_d[:, :].to_broadcast((BL, 1)))
            ident4_s = singles.tile([128, BL], BF16)
            nc.sync.dma_start(out=ident4_s, in_=ident4_d[:, :])
            ones_r128 = singles.tile([1, 128], BF16)
            nc.vector.memset(ones_r128, 1.0)

            h0T = state.tile([128, NK_H, BL], BF16)
            h1T = state.tile([128, NK_H, BL], BF16)
            c0 = state.tile([BL, H], F32)
            c1 = state.tile([BL, H], F32)
            h0blk0 = state.tile([128, NK_H, sb, BL], BF16)
            h0blk1 = state.tile([128, NK_H, sb, BL], BF16)
            xp1blk0 = state.tile([128, sb // 4, NN, 512], BF16)
            xp1blk1 = state.tile([128, sb // 4, NN, 512], BF16)
            h0blk = (h0blk0, h0blk1)
            xp1blk = (xp1blk0, xp1blk1)
            for st in (h0T, h1T, c0, c1):
                nc.vector.memset(st, 0.0)

            def gates_mm(first_mm, kslices, S):
                """Accumulate gates in two [32,1024] psum halves, flush each
                to SBUF S [32,2048] f32 via merged activations.
                Gate order in free dim: i | f | g | o."""
                for half in range(2):
                    gh = pgh.tile([BL, 1024], F32, tag="gh")
                    for n2 in range(2):
                        n = 2 * half + n2
                        o = gh[:, n2 * 512 : (n2 + 1) * 512]
                        first_mm(o, n)
                        ks = kslices(n)
                        for j, (lhsT_k, rhs_k) in enumerate(ks):
                            nc.tensor.matmul(
                                o, lhsT_k, rhs_k, start=False,
                                stop=(j == len(ks) - 1),
                            )
                    if half == 0:
                        nc.scalar.activation(S[:, 0:1024], gh, SIG)
                    else:
                        nc.scalar.activation(S[:, 1024:1536], gh[:, 0:512], TANH)
                        nc.scalar.activation(S[:, 1536:2048], gh[:, 512:1024], SIG)

            def cell_update(hT_dest, cell, S):
                ig = work.tile([BL, 512], F32, tag="ig")
                nc.vector.tensor_mul(ig, S[:, 0:512], S[:, 1024:1536])
                nc.vector.tensor_mul(cell, cell, S[:, 512:1024])
                nc.vector.tensor_add(cell, cell, ig)
                tc_t = work.tile([BL, 512], F32, tag="tc")
                nc.scalar.activation(tc_t, cell, TANH)
                h_new = work.tile([BL, H], BF16, tag="h")
                nc.vector.tensor_mul(h_new, S[:, 1536:2048], tc_t)
                tp = ptp.tile([128, NK_H, BL], BF16, tag="tp")
                for k in range(NK_H):
                    nc.tensor.transpose(
                        tp[:, k, :], h_new[:, k * 128 : (k + 1) * 128],
                        ident4_s[0:BL, :],
                    )
                nc.vector.tensor_copy(hT_dest, tp)

            def l0_step(pb, s):
                t = pb * sb + s
                if s == 0:
                    prev = h0T if pb == 0 else h0blk[(pb - 1) % 2][:, :, sb - 1, :]
                else:
                    prev = h0blk[pb % 2][:, :, s - 1, :]

                def first_mm(o, n):
                    nc.tensor.matmul(
                        o, xt_s[:, t, :], w0a_s[:, n * 512 : (n + 1) * 512],
                        start=True, stop=False,
                    )

                def kslices(n):
                    return [
                        (prev[:, k, :], w0b_s[:, k, n * 512 : (n + 1) * 512])
                        for k in range(NK_H)
                    ]

                S = work.tile([BL, G], F32, tag="S")
                gates_mm(first_mm, kslices, S)
                cell_update(h0blk[pb % 2][:, :, s, :], c0, S)

            def l1_step(db, s):
                sm = (s % 4) * BL
                m = s // 4

                def first_mm(o, n):
                    nc.tensor.matmul(
                        o, ident4_s[sm : sm + BL, :],
                        xp1blk[db % 2][sm : sm + BL, m, n, :],
                        start=True, stop=False, tile_position=(sm, 0),
                    )

                def kslices(n):
                    return [
                        (h1T[:, k, :], w1r_s[:, k, n * 512 : (n + 1) * 512])
                        for k in range(NK_H)
                    ]

                S = work.tile([BL, G], F32, tag="S")
                gates_mm(first_mm, kslices, S)
                cell_update(h1T, c1, S)

            def proj_mtile(db, m):
                for n in range(NN):
                    pp_t = pp.tile([128, 512], F32, tag="pp")
                    nc.tensor.matmul(
                        pp_t, ones_r128, w1b_s[:, n * 512 : (n + 1) * 512],
                        start=True, stop=False,
                    )
                    for k in range(NK_H):
                        nc.tensor.matmul(
                            pp_t,
                            h0blk[db % 2][:, k, 4 * m : 4 * m + 4, :].rearrange(
                                "p a b -> p (a b)"
                            ),
                            w1i_s[:, k, n * 512 : (n + 1) * 512],
                            start=False, stop=(k == NK_H - 1),
                        )
                    nc.scalar.copy(xp1blk[db % 2][:, m, n, :], pp_t)

            for pb in range(nb + 1):
                for s in range(sb):
                    if pb < nb:
                        if pb >= 1 and s % 4 == 0:
                            proj_mtile(pb - 1, s // 4)
                        l0_step(pb, s)
                        if pb >= 1:
                            l1_step(pb - 1, s)
                    else:
                        if s % 4 == 0:
                            proj_mtile(nb - 1, s // 4)
                        l1_step(nb - 1, s)

            fcpt = pp.tile([128, 512], F32, tag="pp")
            fcp = fcpt[0:BL, 0:O]
            for k in range(NK_H):
                nc.tensor.matmul(
                    fcp, h1T[:, k, :], fcw_s[:, k : k + 1],
                    start=(k == 0), stop=(k == NK_H - 1),
                )
            y_s = work.tile([BL, O], F32, tag="y")
            nc.vector.tensor_add(y_s, fcp, fcb_s)
            nc.sync.dma_start(out=y_d[:, :], in_=y_s)

    if split_waits:
        _split_excess_waits(nc)
    return nc


def prep_inputs_v3(x, w_ih_0, w_hh_0, b_ih_0, b_hh_0, w_ih_1, w_hh_1, b_ih_1,
                   b_hh_1, fc_w, fc_b, t_steps: int = T):
    bf = ml_dtypes.bfloat16

    def kchunked(w2dT):
        # [512, 2048] -> [128, 4k, 2048]
        return np.ascontiguousarray(
            w2dT.reshape(NK_H, 128, G).transpose(1, 0, 2)
        )

    w0a = np.concatenate(
        [w_ih_0.T, (b_ih_0 + b_hh_0)[None, :]], axis=0
    ).astype(bf)  # [65, 2048]
    w0b = kchunked(w_hh_0.T).astype(bf)
    w1r = kchunked(w_hh_1.T).astype(bf)
    w1i = kchunked(w_ih_1.T).astype(bf)
    w1b = (b_ih_1 + b_hh_1)[None, :].astype(bf)  # [1, 2048]
    fcw = np.ascontiguousarray(fc_w.reshape(NK_H, 128).T).astype(bf)
    fcb = fc_b.reshape(1, 1).astype(np.float32)
    ident4 = np.concatenate([np.eye(BL, dtype=np.float32)] * 4, axis=0).astype(bf)

    in_maps = []
    for c in range(NCORES):
        xc = x[c * BL : (c + 1) * BL, :t_steps, :]
        xt = np.transpose(xc, (2, 1, 0))
        xt = np.concatenate([xt, np.ones((1, t_steps, BL), np.float32)], axis=0)
        in_maps.append(
            {
                "xt": np.ascontiguousarray(xt).astype(bf),
                "w0a": w0a,
                "w0b": w0b,
                "w1r": w1r,
                "w1i": w1i,
                "w1b": w1b,
                "fcw": fcw,
                "fcb": fcb,
                "ident4": ident4,
            }
        )
    return in_maps


def prep_inputs(x, w_ih_0, w_hh_0, b_ih_0, b_hh_0, w_ih_1, w_hh_1, b_ih_1, b_hh_1,
                fc_w, fc_b, t_steps: int = T):
    """Host-side layout prep + sharding. Returns per-core in_maps."""
    bf = ml_dtypes.bfloat16
    w0a = np.concatenate(
        [w_ih_0.T, (b_ih_0 + b_hh_0)[None, :]], axis=0
    ).astype(bf)  # [65, G]
    w0b = np.ascontiguousarray(
        w_hh_0.T.reshape(NK_H, 128, G).transpose(1, 0, 2)
    ).astype(bf)  # [128, 4, G]
    w1 = np.ascontiguousarray(
        np.concatenate([w_ih_1.T, w_hh_1.T], axis=0)
        .reshape(2 * NK_H, 128, G)
        .transpose(1, 0, 2)
    ).astype(bf)  # [128, 8, G]
    w1bias = (b_ih_1 + b_hh_1)[None, :].astype(bf)  # [1, G]
    fcw = np.ascontiguousarray(fc_w.reshape(NK_H, 128).T).astype(bf)  # [128, 4]
    fcb = fc_b.reshape(1, 1).astype(np.float32)
    ident4 = np.concatenate([np.eye(BL, dtype=np.float32)] * 4, axis=0).astype(bf)

    in_maps = []
    for c in range(NCORES):
        xc = x[c * BL : (c + 1) * BL, :t_steps, :]  # [32, T, 64]
        xt = np.transpose(xc, (2, 1, 0))  # [64, T, 32]
        xt = np.concatenate([xt, np.ones((1, t_steps, BL), np.float32)], axis=0)
        in_maps.append(
            {
                "xt": np.ascontiguousarray(xt).astype(bf),
                "w0a": w0a,
                "w0b": w0b,
                "w1": w1,
                "w1bias": w1bias,
                "fcw": fcw,
                "fcb": fcb,
                "ident4": ident4,
            }
        )
    return in_maps


_NC_CACHE = {}


def kernel(x, w_ih_0, w_hh_0, b_ih_0, b_hh_0, w_ih_1, w_hh_1, b_ih_1, b_hh_1,
           fc_w, fc_b):
    x = np.asarray(x, np.float32)
    args = [np.asarray(a, np.float32) for a in (
        w_ih_0, w_hh_0, b_ih_0, b_hh_0, w_ih_1, w_hh_1, b_ih_1, b_hh_1, fc_w, fc_b)]
    version = _default_version()
    key = (T, version)
    if key not in _NC_CACHE:
        _NC_CACHE[key] = (
            build_lstm_nc_v3(T) if version >= 3 else build_lstm_nc(T, version=version)
        )
    nc = _NC_CACHE[key]
    prep = prep_inputs_v3 if version >= 3 else prep_inputs
    in_maps = prep(x, *args, t_steps=T)
    res = run_bass_kernel_spmd(nc, in_maps, core_ids=list(range(NCORES)))
    return np.concatenate([res.results[c]["y"] for c in range(NCORES)], axis=0)

